# revision 11
# baseline (speedup 1.0000x reference)
"""GAT-style attention score kernel for 8 TRN2 NeuronCores, v6.

Computes out[i,j] = LeakyReLU(Wh[i]@a1 + Wh[j]@a2, slope=0.2) for
N=8192, D=64 -> [8192, 8192] f32 output.

Sharding: output rows across 8 cores ([1024, 8192] slab each).

v6 insight: the ACT engine applies its per-partition bias BEFORE the
table, and the Prelu table honors the alpha operand (HW-verified this
session).  So with s2 pre-broadcast across partitions (host sends
s2b = tile(s2, 128) f16, 2MB) and s1 as a per-partition f32 column,
ONE scalar op computes a whole output block:

    out[p, f] = Prelu(s2b[p, f] + s1c[p])        # fused, 1x, SBUF->SBUF

The Vector engine covers the rest of each tile with a 3-op chain at
packed-f16 rates: ts_add (4x, f32 scalar AP) + ts_mul (4x) +
tt_max (2x) ~= 1 elem/cycle/lane net.

No TensorE, no PSUM, no cross-engine drain hazards.  Per 128-row tile:
S = 2 Prelu ops on cols [0:4096]  (~4.0us)
V = 1 triple on cols [4096:8192]  (~4.3us)
vs the f16 output DMA floor of 5.86us/tile -> purely DMA-bound.

Output leaves as f16 (rel err ~5e-4 vs the 2e-2 gate); host upcasts.
Tile 0 is special-cased into finer ops/pieces for a fast ramp, and the
bulk s2b load rides the idle GpSimd (SWDGE) queue so the scalar queue
only carries the two tiny startup DMAs.
"""

from contextlib import ExitStack

import numpy as np
import concourse.bass as bass
import concourse.mybir as mybir
from concourse.bass_utils import run_bass_kernel_spmd

N = 8192          # nodes
D = 64            # feature dim
M = 8             # cores
ROWS = N // M     # 1024 output rows per core
NT = ROWS // 128  # 8 row tiles of 128 partitions
QW = 2048
SW = 4096         # scalar's columns [0:SW], vector's [SW:N]
NEG_SLOPE = 0.2
NOB = 6           # output tile ring depth

# tile-0 S ops (col ranges) and V triples; later tiles use [0:2048],
# [2048:4096] for S and one [4096:8192] triple for V
S0_OPS = [(0, 1024), (1024, 2048), (2048, 4096)]
V0_OPS = [(4096, 6144), (6144, 8192)]
NS0, NV0 = len(S0_OPS), len(V0_OPS)

_cache = {}


def _so_val(t, j):
    """so count after S op j of tile t completes."""
    return (j + 1) if t == 0 else NS0 + 2 * (t - 1) + j + 1


def _vo_val(t, j=0):
    return (j + 1) if t == 0 else NV0 + (t - 1) + 1


def _build():
    nc = bass.Bass()
    f16 = mybir.dt.float16
    f32 = mybir.dt.float32

    s1c_ext = nc.declare_dram_parameter("s1c", [128, NT], f32, isOutput=False)
    s2b_ext = nc.declare_dram_parameter("s2b", [128, N], f16, isOutput=False)
    out_ext = nc.declare_dram_parameter("out", [ROWS, N], f16, isOutput=True)

    with ExitStack() as ctx:
        sb_s1c = ctx.enter_context(nc.sbuf_tensor("sb_s1c", [128, NT], f32))
        sb_s2b = ctx.enter_context(nc.sbuf_tensor("sb_s2b", [128, N], f16))
        sb_x = ctx.enter_context(nc.sbuf_tensor("sb_x", [128, N - SW], f16))
        sb_m = ctx.enter_context(nc.sbuf_tensor("sb_m", [128, N - SW], f16))
        sb_o = [
            ctx.enter_context(nc.sbuf_tensor(f"sb_o{i}", [128, N], f16))
            for i in range(NOB)
        ]
        sb_junk = ctx.enter_context(nc.sbuf_tensor("sb_junk", [128, 1], f32))
        # one dedicated semaphore per input DMA: a shared counter can hit
        # a threshold via mixed per-engine completions of different DMAs
        din = ctx.enter_context(nc.semaphore("din"))      # s1c + s2b[0:1024]
        dinA = ctx.enter_context(nc.semaphore("dinA"))    # s2b[1024:4096]
        dinB = ctx.enter_context(nc.semaphore("dinB"))    # s2b[4096:6144]
        dinC = ctx.enter_context(nc.semaphore("dinC"))    # s2b[6144:8192]
        so = ctx.enter_context(nc.semaphore("so"))
        vo = ctx.enter_context(nc.semaphore("vo"))
        dt = [ctx.enter_context(nc.semaphore(f"dt{t}")) for t in range(NT)]
        block = ctx.enter_context(nc.Block())

        def dtt(t):  # dt target for tile t
            return 16 * (NS0 + NV0) if t == 0 else 48

        @block.sync
        def _(sync):
            # pure output stream
            for t in range(NT):
                ob = sb_o[t % NOB]
                dst = out_ext[t * 128:(t + 1) * 128, :]
                if t == 0:
                    for j, (lo, hi) in enumerate(S0_OPS):
                        sync.wait_ge(so, _so_val(0, j))
                        sync.dma_start(dst[:, lo:hi], ob[:, lo:hi]).then_inc(dt[0], 16)
                    for j, (lo, hi) in enumerate(V0_OPS):
                        sync.wait_ge(vo, _vo_val(0, j))
                        sync.dma_start(dst[:, lo:hi], ob[:, lo:hi]).then_inc(dt[0], 16)
                else:
                    for j in range(2):
                        sync.wait_ge(so, _so_val(t, j))
                        sync.dma_start(
                            dst[:, j * QW:(j + 1) * QW], ob[:, j * QW:(j + 1) * QW]
                        ).then_inc(dt[t], 16)
                    sync.wait_ge(vo, _vo_val(t))
                    if t == NT - 1:
                        # split the last (vector) piece to shorten the tail
                        sync.dma_start(
                            dst[:, SW:SW + QW], ob[:, SW:SW + QW]
                        ).then_inc(dt[t], 16)
                        sync.dma_start(
                            dst[:, SW + QW:N], ob[:, SW + QW:N]
                        ).then_inc(dt[t], 16)
                    else:
                        sync.dma_start(dst[:, SW:N], ob[:, SW:N]).then_inc(dt[t], 16)

        @block.gpsimd
        def _(gpsimd):
            # bulk s2b load on the otherwise-idle SWDGE queue
            gpsimd.dma_start(
                sb_s2b[:, 1024:SW], s2b_ext[:, 1024:SW]
            ).then_inc(dinA, 16)
            gpsimd.dma_start(
                sb_s2b[:, SW:SW + QW], s2b_ext[:, SW:SW + QW]
            ).then_inc(dinB, 16)
            gpsimd.dma_start(
                sb_s2b[:, SW + QW:N], s2b_ext[:, SW + QW:N]
            ).then_inc(dinC, 16)

        @block.scalar
        def _(scalar):
            scalar.dma_start(sb_s1c[:, :], s1c_ext[:, :]).then_inc(din, 16)
            scalar.dma_start(
                sb_s2b[:, 0:1024], s2b_ext[:, 0:1024]
            ).then_inc(din, 16)
            # warm the Prelu table while they fly
            scalar.activation(
                sb_junk[:, :], sb_junk[:, :],
                mybir.ActivationFunctionType.Prelu,
                bias=0.0, scale=1.0, alpha=NEG_SLOPE,
            )
            for t in range(NT):
                ob = sb_o[t % NOB]
                b = sb_s1c[:, t:t + 1]
                if t >= NOB:
                    scalar.wait_ge(dt[t - NOB], dtt(t - NOB))
                ops = S0_OPS if t == 0 else [(0, QW), (QW, SW)]
                for j, (lo, hi) in enumerate(ops):
                    if t == 0:
                        if j == 0:
                            scalar.wait_ge(din, 32)
                        elif j == 1:
                            scalar.wait_ge(dinA, 16)
                    scalar.activation(
                        ob[:, lo:hi], sb_s2b[:, lo:hi],
                        mybir.ActivationFunctionType.Prelu,
                        bias=b, scale=1.0, alpha=NEG_SLOPE,
                    ).then_inc(so)

        @block.vector
        def _(vector):
            for t in range(NT):
                ob = sb_o[t % NOB]
                b = sb_s1c[:, t:t + 1]
                if t >= NOB:
                    vector.wait_ge(dt[t - NOB], dtt(t - NOB))
                ops = V0_OPS if t == 0 else [(SW, N)]
                for j, (lo, hi) in enumerate(ops):
                    if t == 0:
                        vector.wait_ge(din, 32)                  # s1c landed
                        vector.wait_ge(dinB if j == 0 else dinC, 16)
                    x = sb_x[:, lo - SW:hi - SW]
                    m = sb_m[:, lo - SW:hi - SW]
                    vector.tensor_scalar_add(x, sb_s2b[:, lo:hi], b)
                    vector.tensor_scalar_mul(m, x, NEG_SLOPE)
                    vector.tensor_max(ob[:, lo:hi], x, m).then_inc(vo)

    return nc


def _run(Wh, a, trace=False, **kw):
    Wh = np.ascontiguousarray(np.asarray(Wh, dtype=np.float32))
    a = np.ascontiguousarray(np.asarray(a, dtype=np.float32))
    assert Wh.shape == (N, D) and a.shape == (2 * D, 1)

    if "nc" not in _cache:
        _cache["nc"] = _build()
    nc = _cache["nc"]

    s1 = Wh @ a[:D, 0]                         # [N] f32 row contribution
    s2b = np.ascontiguousarray(
        np.broadcast_to((Wh @ a[D:, 0]).astype(np.float16), (128, N))
    )

    in_maps = []
    for i in range(M):
        sl = s1[i * ROWS:(i + 1) * ROWS]
        s1c = np.ascontiguousarray(sl.reshape(NT, 128).T.astype(np.float32))
        in_maps.append({"s1c": s1c, "s2b": s2b})

    res = run_bass_kernel_spmd(nc, in_maps, core_ids=list(range(M)), trace=trace, **kw)
    out = np.concatenate(
        [res.results[i]["out"].astype(np.float32) for i in range(M)], axis=0
    )
    return out, res


def kernel(Wh, a):
    return _run(Wh, a)[0]


# revision 12
# speedup vs baseline: 1.0014x; 1.0014x over previous
"""GAT-style attention score kernel for 8 TRN2 NeuronCores, v7.

Computes out[i,j] = LeakyReLU(Wh[i]@a1 + Wh[j]@a2, slope=0.2) for
N=8192, D=64 -> [8192, 8192] f32 output.

Sharding: output rows across 8 cores ([1024, 8192] slab each).

Core idea (HW-verified): the ACT engine applies its per-partition bias
BEFORE the activation table and the Prelu table honors the alpha
operand, so with s2 pre-broadcast (host sends s2b = tile(s2,128) f16)
and s1 as an f32 per-partition column, one fused scalar op computes a
whole output block:

    out[p, f] = Prelu(s2b[p, f] + s1c[p])      # evac+bias+leaky, 1x

Vector covers the rest of each 128-row tile with ts_add (4x, f32
scalar AP) + ts_mul (4x) + tt_max (2x), all f16 SBUF.

No TensorE, no PSUM.  Per tile: S = 2 Prelus on cols [0:4736]
(~4.5us), V = 1 triple on [4736:8192] (~3.8us), both under the
measured ~4.9us/tile f16 output stream rate -> DMA-bound.

Startup choreography (the 2MB s2b load is the ramp constraint):
 - scalar HWDGE queue: s2b[0:1024], s1c, s2b[4736:6464] (small, feed
   the first S op and first V triple), then the Prelu table warm.
 - gpsimd SWDGE queue: the bulk (s2b[1024:4736], s2b[6464:8192]),
   gated on the first chunk landing so it can't starve the critical
   path (all DMAs share the 16 SDMA engines).
 - every input DMA has a DEDICATED semaphore: a shared counter can
   reach a threshold via mixed per-engine completions of different
   DMAs (this exact bug corrupted one core in two earlier versions).
 - tile 0 is split finer and its DMA pieces are emitted in expected
   readiness order (the sync queue is FIFO; a stalled head blocks
   later-ready pieces).

Output leaves as f16 (rel err ~3e-4 vs the 2e-2 gate); host upcasts.
"""

from contextlib import ExitStack

import numpy as np
import concourse.bass as bass
import concourse.mybir as mybir
from concourse.bass_utils import run_bass_kernel_spmd

N = 8192          # nodes
D = 64            # feature dim
M = 8             # cores
ROWS = N // M     # 1024 output rows per core
NT = ROWS // 128  # 8 row tiles of 128 partitions
SW = 4736         # scalar's columns [0:SW], vector's [SW:N]
SH = SW // 2      # 2368, scalar op width
VMID = SW + (N - SW) // 2  # 6464, vector tile-0 / tail split
NEG_SLOPE = 0.2
NOB = 6           # output tile ring depth

S0_OPS = [(0, 1024), (1024, SH), (SH, SW)]
V0_OPS = [(SW, VMID), (VMID, N)]
NS0, NV0 = len(S0_OPS), len(V0_OPS)

_cache = {}


def _so_val(t, j):
    return (j + 1) if t == 0 else NS0 + 2 * (t - 1) + j + 1


def _vo_val(t, j=0):
    return (j + 1) if t == 0 else NV0 + (t - 1) + 1


def _build():
    nc = bass.Bass()
    f16 = mybir.dt.float16
    f32 = mybir.dt.float32

    s1c_ext = nc.declare_dram_parameter("s1c", [128, NT], f32, isOutput=False)
    s2b_ext = nc.declare_dram_parameter("s2b", [128, N], f16, isOutput=False)
    out_ext = nc.declare_dram_parameter("out", [ROWS, N], f16, isOutput=True)

    with ExitStack() as ctx:
        sb_s1c = ctx.enter_context(nc.sbuf_tensor("sb_s1c", [128, NT], f32))
        sb_s2b = ctx.enter_context(nc.sbuf_tensor("sb_s2b", [128, N], f16))
        sb_x = ctx.enter_context(nc.sbuf_tensor("sb_x", [128, N - SW], f16))
        sb_m = ctx.enter_context(nc.sbuf_tensor("sb_m", [128, N - SW], f16))
        sb_o = [
            ctx.enter_context(nc.sbuf_tensor(f"sb_o{i}", [128, N], f16))
            for i in range(NOB)
        ]
        sb_junk = ctx.enter_context(nc.sbuf_tensor("sb_junk", [128, 1], f32))
        dQ0 = ctx.enter_context(nc.semaphore("dQ0"))    # s2b[0:1024]
        dS1 = ctx.enter_context(nc.semaphore("dS1"))    # s1c
        dinA = ctx.enter_context(nc.semaphore("dinA"))  # s2b[1024:4736]
        dinB = ctx.enter_context(nc.semaphore("dinB"))  # s2b[4736:6464]
        dinC = ctx.enter_context(nc.semaphore("dinC"))  # s2b[6464:8192]
        so = ctx.enter_context(nc.semaphore("so"))
        vo = ctx.enter_context(nc.semaphore("vo"))
        dt = [ctx.enter_context(nc.semaphore(f"dt{t}")) for t in range(NT)]
        block = ctx.enter_context(nc.Block())

        def dtt(t):
            return 16 * (NS0 + NV0) if t == 0 else (64 if t == NT - 1 else 48)

        @block.sync
        def _(sync):
            for t in range(NT):
                ob = sb_o[t % NOB]
                dst = out_ext[t * 128:(t + 1) * 128, :]
                if t == 0:
                    # readiness order: S[0:1024], V[4736:6464], S[1024:2368],
                    # S[2368:4736], V[6464:8192]
                    pieces = [
                        (0, 1024, so, _so_val(0, 0)),
                        (SW, VMID, vo, _vo_val(0, 0)),
                        (1024, SH, so, _so_val(0, 1)),
                        (SH, SW, so, _so_val(0, 2)),
                        (VMID, N, vo, _vo_val(0, 1)),
                    ]
                    for lo, hi, sem, val in pieces:
                        sync.wait_ge(sem, val)
                        sync.dma_start(dst[:, lo:hi], ob[:, lo:hi]).then_inc(dt[0], 16)
                else:
                    for j in range(2):
                        sync.wait_ge(so, _so_val(t, j))
                        sync.dma_start(
                            dst[:, j * SH:(j + 1) * SH], ob[:, j * SH:(j + 1) * SH]
                        ).then_inc(dt[t], 16)
                    sync.wait_ge(vo, _vo_val(t))
                    if t == NT - 1:
                        # split the last piece to shorten the tail
                        sync.dma_start(
                            dst[:, SW:VMID], ob[:, SW:VMID]
                        ).then_inc(dt[t], 16)
                        sync.dma_start(
                            dst[:, VMID:N], ob[:, VMID:N]
                        ).then_inc(dt[t], 16)
                    else:
                        sync.dma_start(dst[:, SW:N], ob[:, SW:N]).then_inc(dt[t], 16)

        @block.gpsimd
        def _(gpsimd):
            # bulk s2b on the idle SWDGE queue, deferred behind the
            # critical first chunk
            gpsimd.wait_ge(dQ0, 16)
            gpsimd.dma_start(
                sb_s2b[:, 1024:SW], s2b_ext[:, 1024:SW]
            ).then_inc(dinA, 16)
            gpsimd.dma_start(
                sb_s2b[:, VMID:N], s2b_ext[:, VMID:N]
            ).then_inc(dinC, 16)

        @block.scalar
        def _(scalar):
            scalar.dma_start(
                sb_s2b[:, 0:1024], s2b_ext[:, 0:1024]
            ).then_inc(dQ0, 16)
            scalar.dma_start(sb_s1c[:, :], s1c_ext[:, :]).then_inc(dS1, 16)
            scalar.dma_start(
                sb_s2b[:, SW:VMID], s2b_ext[:, SW:VMID]
            ).then_inc(dinB, 16)
            # warm the Prelu table while they fly
            scalar.activation(
                sb_junk[:, :], sb_junk[:, :],
                mybir.ActivationFunctionType.Prelu,
                bias=0.0, scale=1.0, alpha=NEG_SLOPE,
            )
            for t in range(NT):
                ob = sb_o[t % NOB]
                b = sb_s1c[:, t:t + 1]
                if t >= NOB:
                    scalar.wait_ge(dt[t - NOB], dtt(t - NOB))
                ops = S0_OPS if t == 0 else [(0, SH), (SH, SW)]
                for j, (lo, hi) in enumerate(ops):
                    if t == 0:
                        if j == 0:
                            scalar.wait_ge(dQ0, 16)
                            scalar.wait_ge(dS1, 16)
                        elif j == 1:
                            scalar.wait_ge(dinA, 16)
                    scalar.activation(
                        ob[:, lo:hi], sb_s2b[:, lo:hi],
                        mybir.ActivationFunctionType.Prelu,
                        bias=b, scale=1.0, alpha=NEG_SLOPE,
                    ).then_inc(so)

        @block.vector
        def _(vector):
            for t in range(NT):
                ob = sb_o[t % NOB]
                b = sb_s1c[:, t:t + 1]
                if t >= NOB:
                    vector.wait_ge(dt[t - NOB], dtt(t - NOB))
                ops = V0_OPS if t == 0 else [(SW, N)]
                for j, (lo, hi) in enumerate(ops):
                    if t == 0:
                        if j == 0:
                            vector.wait_ge(dS1, 16)
                            vector.wait_ge(dinB, 16)
                        else:
                            vector.wait_ge(dinC, 16)
                    x = sb_x[:, lo - SW:hi - SW]
                    m = sb_m[:, lo - SW:hi - SW]
                    vector.tensor_scalar_add(x, sb_s2b[:, lo:hi], b)
                    vector.tensor_scalar_mul(m, x, NEG_SLOPE)
                    vector.tensor_max(ob[:, lo:hi], x, m).then_inc(vo)

    return nc


def _run(Wh, a, trace=False, **kw):
    Wh = np.ascontiguousarray(np.asarray(Wh, dtype=np.float32))
    a = np.ascontiguousarray(np.asarray(a, dtype=np.float32))
    assert Wh.shape == (N, D) and a.shape == (2 * D, 1)

    if "nc" not in _cache:
        _cache["nc"] = _build()
    nc = _cache["nc"]

    s1 = Wh @ a[:D, 0]                         # [N] f32 row contribution
    s2b = np.ascontiguousarray(
        np.broadcast_to((Wh @ a[D:, 0]).astype(np.float16), (128, N))
    )

    in_maps = []
    for i in range(M):
        sl = s1[i * ROWS:(i + 1) * ROWS]
        s1c = np.ascontiguousarray(sl.reshape(NT, 128).T.astype(np.float32))
        in_maps.append({"s1c": s1c, "s2b": s2b})

    res = run_bass_kernel_spmd(nc, in_maps, core_ids=list(range(M)), trace=trace, **kw)
    out = np.concatenate(
        [res.results[i]["out"].astype(np.float32) for i in range(M)], axis=0
    )
    return out, res


def kernel(Wh, a):
    return _run(Wh, a)[0]
